# revision 8
# baseline (speedup 1.0000x reference)
"""Bass/Tile Trainium2 kernel for nn_CrossAttention (B=4, Nq=Nk=2048, D=1024, H=16).

Sharding: 8 cores; core c handles batch b=c//2, query rows [(c%2)*1024, (c%2+1)*1024).
Each core runs the full per-slice cross-attention:
  qh = q @ Wq.T ; kh = k @ Wk.T ; vh = v @ Wv.T   (per-head, transposed layouts)
  scores_T[k, q] = kh_T.T-style PE matmuls; masked softmax via exp(s*SCALE + maskbias)
  av_T[d, q] = vh_aug.T @ E_T  (ones column gives softmax denominators)
  out = (av_T / denom) @ Wp.T + bp

All matmul operands bf16 (fp32 PSUM accumulation); final output fp32.
"""
import numpy as np
import ml_dtypes

import concourse.bass as bass
import concourse.mybir as mybir
import concourse.tile as tile
from concourse import bacc
from concourse.bass_utils import run_bass_kernel_spmd

F32 = mybir.dt.float32
BF16 = mybir.dt.bfloat16
NPBF16 = ml_dtypes.bfloat16

B, NQ_FULL, NK_FULL, D, H, DH = 4, 2048, 2048, 1024, 16, 64
SCALE = DH ** -0.5
MASK_NEG = -60.0  # additive bias (post-scale) for masked keys; exp(-60) ~ 9e-27
N_CORES = 8


def build_nc(nq, nk, d=D, h=H):
    """Build the per-core Bass program. nq = q rows per core, nk = key rows."""
    dh = d // h
    assert dh == 64 and d % 128 == 0
    IC = d // 128          # contraction chunks for projections
    OC = d // 128          # output chunks (128 rows each)
    NKT = nk // 128        # key tiles
    QC = max(1, nq // 512)  # q chunks of <=512
    QW = min(nq, 512)      # q chunk width
    KC = max(1, nk // 512)  # k chunks of <=512 (for K-projection free dim)
    KW = min(nk, 512)
    HP = h // 2            # head pairs

    nc = bacc.Bacc("TRN2", target_bir_lowering=False, debug=False)

    xq = nc.declare_dram_parameter("xq", [nq, d], BF16, isOutput=False)
    xk = nc.declare_dram_parameter("xk", [nk, d], BF16, isOutput=False)
    xv = nc.declare_dram_parameter("xv", [nk, d], BF16, isOutput=False)
    wq = nc.declare_dram_parameter("wq", [d, d], BF16, isOutput=False)  # Wq.T [in, out]
    wk = nc.declare_dram_parameter("wk", [d, d], BF16, isOutput=False)
    wv = nc.declare_dram_parameter("wv", [d, d], BF16, isOutput=False)
    wp = nc.declare_dram_parameter("wp", [d, d], BF16, isOutput=False)
    maskb = nc.declare_dram_parameter("maskb", [128, NKT], F32, isOutput=False)
    bpb = nc.declare_dram_parameter("bpb", [1, d], F32, isOutput=False)
    out = nc.declare_dram_parameter("out", [nq, d], F32, isOutput=True)

    with tile.TileContext(nc) as tc:
        with (
            tc.tile_pool(name="wpool", bufs=1) as wpool,
            tc.tile_pool(name="const", bufs=1) as cpool,
            tc.tile_pool(name="acts", bufs=1) as apool,
        ):
            # --- constants ---
            maskb_s = cpool.tile([128, NKT], F32, tag="maskb")
            nc.sync.dma_start(out=maskb_s[:, :], in_=maskb[:, :])
            bp_row = cpool.tile([1, d], F32, tag="bp_row")
            nc.sync.dma_start(out=bp_row[:, :], in_=bpb[:, :])
            bp_s = cpool.tile([128, d], F32, tag="bp")
            nc.gpsimd.partition_broadcast(bp_s[:, :], bp_row[:, :])

            # --- persistent activations ---
            qh_s = apool.tile([128, OC, nq], BF16, tag="qh")     # qh_T
            kh_s = apool.tile([128, OC, nk], BF16, tag="kh")     # kh_T
            vh_s = apool.tile([128, NKT, h, dh + 1], BF16, tag="vh")  # vh + ones col
            avT_s = apool.tile([128, OC, nq], BF16, tag="avT")   # normalized att out, T

            nc.vector.memset(vh_s[:, :, :, dh:dh + 1], 1.0)

            # ============ load weights + transposed activations, project ============
            with (
                tc.tile_pool(name="xT", bufs=1) as xpool,
                tc.tile_pool(name="mm_ps", bufs=3, space="PSUM") as mmps,
            ):
                def load_w(wdram):
                    w_s = wpool.tile([128, IC, d], BF16, tag="W")
                    nc.sync.dma_start(
                        out=w_s[:, :, :],
                        in_=wdram.ap().rearrange("(c p) o -> p c o", p=128),
                    )
                    return w_s

                def load_xT(xdram, n):
                    xT = xpool.tile([128, IC, n], BF16, tag="xT")
                    for ic in range(IC):
                        nc.sync.dma_start_transpose(
                            out=xT[:, ic, :], in_=xdram[:, ic * 128:(ic + 1) * 128]
                        )
                    return xT

                def project(w_s, xT, n, emit):
                    """out_chunk[oc][128, n] = sum_ic w_s[:,ic,oc*128:+128].T @ xT[:,ic,:]
                    emit(oc, j0, jw, psum_ap) consumes each [128, <=512] result."""
                    nchunks = max(1, n // 512)
                    w_ = min(n, 512)
                    for oc in range(OC):
                        for j in range(nchunks):
                            ps = mmps.tile([128, 512], F32, tag="ps")
                            for ic in range(IC):
                                nc.tensor.matmul(
                                    ps[:, :w_],
                                    w_s[:, ic, oc * 128:(oc + 1) * 128],
                                    xT[:, ic, j * w_:(j + 1) * w_],
                                    start=(ic == 0),
                                    stop=(ic == IC - 1),
                                )
                            emit(oc, j * w_, w_, ps)

                # Q projection -> qh_s
                wq_s = load_w(wq)
                xqT = load_xT(xq, nq)
                project(
                    wq_s, xqT, nq,
                    lambda oc, j0, jw, ps: nc.vector.tensor_copy(
                        qh_s[:, oc, j0:j0 + jw], ps[:, :jw]
                    ),
                )
                # K projection -> kh_s
                wk_s = load_w(wk)
                xkT = load_xT(xk, nk)
                project(
                    wk_s, xkT, nk,
                    lambda oc, j0, jw, ps: nc.vector.tensor_copy(
                        kh_s[:, oc, j0:j0 + jw], ps[:, :jw]
                    ),
                )
                # V projection -> vh_s (k on partitions: lhsT = xvT chunk, rhs = w chunk)
                wv_s = load_w(wv)
                xvT = load_xT(xv, nk)
                for kt in range(NKT):
                    for oc2 in range(d // 512):
                        ps = mmps.tile([128, 512], F32, tag="ps")
                        for ic in range(IC):
                            nc.tensor.matmul(
                                ps[:, :],
                                xvT[:, ic, kt * 128:(kt + 1) * 128],
                                wv_s[:, ic, oc2 * 512:(oc2 + 1) * 512],
                                start=(ic == 0),
                                stop=(ic == IC - 1),
                            )
                        h0 = oc2 * (512 // dh)
                        nc.vector.tensor_copy(
                            vh_s[:, kt, h0:h0 + 512 // dh, 0:dh],
                            ps[:, :].rearrange("p (a b) -> p a b", b=dh),
                        )

            wp_s = load_w(wp)  # wpool still open

            # ============ attention ============
            with (
                tc.tile_pool(name="epool", bufs=3) as epool,
                tc.tile_pool(name="sc_ps", bufs=4, space="PSUM") as scps,
                tc.tile_pool(name="av_ps", bufs=2, space="PSUM") as avps,
                tc.tile_pool(name="rpool", bufs=2) as rpool,
                tc.tile_pool(name="avn", bufs=2) as avnpool,
            ):
                for hp in range(HP):
                    for j in range(QC):
                        es = [
                            epool.tile([128, NKT, QW], BF16, tag="e", name=f"e{hp}_{j}_{hf}")
                            for hf in range(2)
                        ]
                        for kt in range(NKT):
                            for half, e in enumerate(es):
                                p0 = half * 64
                                ps = scps.tile([128, 512], F32, tag="sc")
                                nc.tensor.matmul(
                                    ps[:, :QW],
                                    kh_s[p0:p0 + 64, hp, kt * 128:(kt + 1) * 128],
                                    qh_s[p0:p0 + 64, hp, j * QW:(j + 1) * QW],
                                    start=True, stop=True,
                                )
                                nc.scalar.activation(
                                    e[:, kt, :], ps[:, :QW],
                                    mybir.ActivationFunctionType.Exp,
                                    bias=maskb_s[:, kt:kt + 1], scale=SCALE,
                                )
                        for half, e in enumerate(es):
                            hh = 2 * hp + half
                            av = avps.tile([dh + 1, 512], F32, tag="av")
                            for kt in range(NKT):
                                nc.tensor.matmul(
                                    av[:, :QW],
                                    vh_s[:, kt, hh, :],
                                    e[:, kt, :],
                                    start=(kt == 0), stop=(kt == NKT - 1),
                                )
                            rec = rpool.tile([dh + 1, QW], F32, tag="rec")
                            nc.vector.reciprocal(rec[dh:dh + 1, :], av[dh:dh + 1, :QW])
                            r0 = rpool.tile([1, QW], F32, tag="r0")
                            nc.sync.dma_start(out=r0[:, :], in_=rec[dh:dh + 1, :])
                            rb = rpool.tile([dh, QW], F32, tag="rb")
                            nc.gpsimd.partition_broadcast(rb[:, :], r0[:, :])
                            if half == 0:
                                nc.vector.tensor_mul(
                                    avT_s[0:dh, hp, j * QW:(j + 1) * QW],
                                    av[0:dh, :QW], rb[:, :],
                                )
                            else:
                                avn = avnpool.tile([dh, QW], BF16, tag="avn")
                                nc.vector.tensor_mul(avn[:, :], av[0:dh, :QW], rb[:, :])
                                nc.sync.dma_start(
                                    out=avT_s[64:128, hp, j * QW:(j + 1) * QW],
                                    in_=avn[:, :],
                                )

            # ============ output projection ============
            with (
                tc.tile_pool(name="o_ps", bufs=3, space="PSUM") as ops,
                tc.tile_pool(name="obuf", bufs=3) as obuf,
            ):
                for qt in range(nq // 128):
                    for oc2 in range(d // 512):
                        ps = ops.tile([128, 512], F32, tag="o")
                        for dc in range(OC):
                            nc.tensor.matmul(
                                ps[:, :],
                                avT_s[:, dc, qt * 128:(qt + 1) * 128],
                                wp_s[:, dc, oc2 * 512:(oc2 + 1) * 512],
                                start=(dc == 0), stop=(dc == OC - 1),
                            )
                        ot = obuf.tile([128, 512], F32, tag="ot")
                        nc.vector.tensor_add(
                            ot[:, :], ps[:, :], bp_s[:, oc2 * 512:(oc2 + 1) * 512]
                        )
                        nc.sync.dma_start(
                            out=out[qt * 128:(qt + 1) * 128, oc2 * 512:(oc2 + 1) * 512],
                            in_=ot[:, :],
                        )

    nc.compile()
    return nc


def host_prep(q, k, v, attention_mask, Wq, Wk, Wv, Wp, bp, nq_per_core=None):
    """Slice + cast full inputs into per-core input maps."""
    nq = nq_per_core or (NQ_FULL * B // N_CORES)
    nk = k.shape[1]
    nkt = nk // 128
    cores_per_b = N_CORES // B
    wqT = np.ascontiguousarray(Wq.T).astype(NPBF16)
    wkT = np.ascontiguousarray(Wk.T).astype(NPBF16)
    wvT = np.ascontiguousarray(Wv.T).astype(NPBF16)
    wpT = np.ascontiguousarray(Wp.T).astype(NPBF16)
    bpb = np.ascontiguousarray(bp[None, :]).astype(np.float32)
    in_maps = []
    for c in range(N_CORES):
        b, qi = divmod(c, cores_per_b)
        mb = np.where(attention_mask[b] != 0, 0.0, MASK_NEG).astype(np.float32)
        in_maps.append({
            "xq": np.ascontiguousarray(q[b, qi * nq:(qi + 1) * nq]).astype(NPBF16),
            "xk": np.ascontiguousarray(k[b]).astype(NPBF16),
            "xv": np.ascontiguousarray(v[b]).astype(NPBF16),
            "wq": wqT, "wk": wkT, "wv": wvT, "wp": wpT,
            "maskb": np.ascontiguousarray(mb.reshape(nkt, 128).T),
            "bpb": bpb,
        })
    return in_maps


_NC_CACHE = {}


def get_nc(nq, nk):
    key = (nq, nk)
    if key not in _NC_CACHE:
        _NC_CACHE[key] = build_nc(nq, nk)
    return _NC_CACHE[key]


def kernel(q, k, v, attention_mask, Wq, Wk, Wv, Wp, bp):
    nq = NQ_FULL * B // N_CORES
    in_maps = host_prep(q, k, v, attention_mask, Wq, Wk, Wv, Wp, bp)
    nc = get_nc(nq, NK_FULL)
    res = run_bass_kernel_spmd(nc, in_maps, core_ids=list(range(N_CORES)))
    cores_per_b = N_CORES // B
    out = np.empty((B, NQ_FULL, D), np.float32)
    for c in range(N_CORES):
        b, qi = divmod(c, cores_per_b)
        out[b, qi * nq:(qi + 1) * nq] = res.results[c]["out"]
    return out


# revision 10
# speedup vs baseline: 1.8527x; 1.8527x over previous
"""Bass/Tile Trainium2 kernel for nn_CrossAttention (B=4, Nq=Nk=2048, D=1024, H=16).

Sharding: 8 cores; core c handles batch b=c//2, query rows [(c%2)*1024, (c%2+1)*1024).

Ragged-sequence optimization: valid keys (attention_mask==1) are packed on the host,
so the kernel only attends over ~Nk/2 keys; pad rows get a -60 additive bias before
exp (softmax over packed keys == masked softmax over the full set).

Per-core pipeline (all matmul operands bf16, fp32 PSUM accumulation):
  qh_T = Wq @ q.T            (DMA-transposed q, upfront)
  per head-pair hp (K/V projections double-buffered to overlap with attention):
    kh_T pair = Wk-chunk @ k.T ; vh pair = [v @ Wv-chunk | ones-block]
    scores_T[k, q] per 128-k tile; E = exp(scores*SCALE + maskbias[k])
    av[128, q] = vh_aug.T @ E  -> rows 0-63 value, rows 64-127 softmax denominator
    avT = av[0:64] * recip(av[64:128])
  out = avT_all @ Wp.T + bp
"""
import numpy as np
import ml_dtypes

import concourse.bass as bass
import concourse.mybir as mybir
import concourse.tile as tile
from concourse import bacc
from concourse.bass_utils import run_bass_kernel_spmd

F32 = mybir.dt.float32
BF16 = mybir.dt.bfloat16
NPBF16 = ml_dtypes.bfloat16

B, NQ_FULL, NK_FULL, D, H, DH = 4, 2048, 2048, 1024, 16, 64
SCALE = DH ** -0.5
MASK_NEG = -60.0  # additive bias (post-scale) for pad keys; exp(-60) ~ 9e-27
N_CORES = 8


def _chunks(n, w=512):
    out, j = [], 0
    while j < n:
        out.append((j, min(w, n - j)))
        j += min(w, n - j)
    return out


def build_nc(nq, nk, d=D, h=H):
    """Build the per-core Bass program. nq = q rows per core, nk = packed key rows."""
    dh = d // h
    assert dh == 64 and d % 128 == 0 and nk % 128 == 0
    IC = d // 128          # contraction chunks for projections
    OC = d // 128          # output chunks (128 rows each)
    NKT = nk // 128        # key tiles
    QC = max(1, nq // 512)  # q chunks of <=512
    QW = min(nq, 512)      # q chunk width
    HP = h // 2            # head pairs

    nc = bacc.Bacc("TRN2", target_bir_lowering=False, debug=False)

    xq = nc.declare_dram_parameter("xq", [nq, d], BF16, isOutput=False)
    xk = nc.declare_dram_parameter("xk", [nk, d], BF16, isOutput=False)
    xv = nc.declare_dram_parameter("xv", [nk, d], BF16, isOutput=False)
    wq = nc.declare_dram_parameter("wq", [d, d], BF16, isOutput=False)  # Wq.T [in, out]
    wk = nc.declare_dram_parameter("wk", [d, d], BF16, isOutput=False)
    wv = nc.declare_dram_parameter("wv", [d, d], BF16, isOutput=False)
    wp = nc.declare_dram_parameter("wp", [d, d], BF16, isOutput=False)
    maskb = nc.declare_dram_parameter("maskb", [128, NKT], F32, isOutput=False)
    bpb = nc.declare_dram_parameter("bpb", [1, d], F32, isOutput=False)
    out = nc.declare_dram_parameter("out", [nq, d], F32, isOutput=True)

    with tile.TileContext(nc) as tc:
        with (
            tc.tile_pool(name="wpool", bufs=3) as wpool,
            tc.tile_pool(name="const", bufs=1) as cpool,
            tc.tile_pool(name="acts", bufs=1) as apool,
            tc.tile_pool(name="xT", bufs=1) as xpool,
            tc.tile_pool(name="mm_ps", bufs=2, space="PSUM") as mmps,
            tc.tile_pool(name="kvpair", bufs=2) as kvpool,
        ):
            # --- constants ---
            maskb_s = cpool.tile([128, NKT], F32, tag="maskb")
            nc.sync.dma_start(out=maskb_s[:, :], in_=maskb[:, :])
            bp_row = cpool.tile([1, d], F32, tag="bp_row")
            nc.sync.dma_start(out=bp_row[:, :], in_=bpb[:, :])
            bp_s = cpool.tile([128, d], F32, tag="bp")
            nc.gpsimd.partition_broadcast(bp_s[:, :], bp_row[:, :])

            # --- persistent activations ---
            qh_s = apool.tile([128, OC, nq], BF16, tag="qh")     # qh_T
            avT_s = apool.tile([128, OC, nq], BF16, tag="avT")   # normalized att out, T

            def load_w(wdram, name):
                w_s = wpool.tile([128, IC, d], BF16, tag="W", name=name)
                nc.sync.dma_start(
                    out=w_s[:, :, :],
                    in_=wdram.ap().rearrange("(c p) o -> p c o", p=128),
                )
                return w_s

            def load_xT(xdram, n, name):
                xT = xpool.tile([128, IC, n], BF16, tag=name, name=name)
                for ic in range(IC):
                    nc.sync.dma_start_transpose(
                        out=xT[:, ic, :], in_=xdram[:, ic * 128:(ic + 1) * 128]
                    )
                return xT

            # ---- Q projection (upfront, dense PE work) ----
            wq_s = load_w(wq, "wq_s")
            xqT = load_xT(xq, nq, "xqT")
            for oc in range(OC):
                for j0, jw in _chunks(nq):
                    ps = mmps.tile([128, 512], F32, tag="ps")
                    for ic in range(IC):
                        nc.tensor.matmul(
                            ps[:, :jw],
                            wq_s[:, ic, oc * 128:(oc + 1) * 128],
                            xqT[:, ic, j0:j0 + jw],
                            start=(ic == 0), stop=(ic == IC - 1),
                        )
                    nc.vector.tensor_copy(qh_s[:, oc, j0:j0 + jw], ps[:, :jw])

            wk_s = load_w(wk, "wk_s")
            wv_s = load_w(wv, "wv_s")
            xkT = load_xT(xk, nk, "xkT")
            xvT = load_xT(xv, nk, "xvT")

            # ---- head-pair loop: K/V projection + attention ----
            with (
                tc.tile_pool(name="epool", bufs=4) as epool,
                tc.tile_pool(name="sc_ps", bufs=4, space="PSUM") as scps,
                tc.tile_pool(name="av_ps", bufs=2, space="PSUM") as avps,
                tc.tile_pool(name="rpool", bufs=3) as rpool,
                tc.tile_pool(name="avn", bufs=2) as avnpool,
            ):
                for hp in range(HP):
                    # K projection for this pair -> kh_p [128, nk]
                    kh_p = kvpool.tile([128, nk], BF16, tag="kh", name=f"kh{hp}")
                    for j0, jw in _chunks(nk):
                        ps = mmps.tile([128, 512], F32, tag="ps", name=f"kps{hp}_{j0}")
                        for ic in range(IC):
                            nc.tensor.matmul(
                                ps[:, :jw],
                                wk_s[:, ic, hp * 128:(hp + 1) * 128],
                                xkT[:, ic, j0:j0 + jw],
                                start=(ic == 0), stop=(ic == IC - 1),
                            )
                        nc.vector.tensor_copy(kh_p[:, j0:j0 + jw], ps[:, :jw])
                    # V projection -> vh_p [128, NKT, 2, 128] ([vh | ones])
                    vh_p = kvpool.tile([128, NKT, 2, 128], BF16, tag="vh", name=f"vh{hp}")
                    nc.gpsimd.memset(vh_p[:, :, :, dh:], 1.0)
                    for kt in range(NKT):
                        ps = mmps.tile([128, 128], F32, tag="ps", name=f"vps{hp}_{kt}")
                        for ic in range(IC):
                            nc.tensor.matmul(
                                ps[:, :],
                                xvT[:, ic, kt * 128:(kt + 1) * 128],
                                wv_s[:, ic, hp * 128:(hp + 1) * 128],
                                start=(ic == 0), stop=(ic == IC - 1),
                            )
                        nc.vector.tensor_copy(
                            vh_p[:, kt, :, 0:dh],
                            ps[:, :].rearrange("p (a b) -> p a b", b=dh),
                        )
                    # attention for both heads of the pair
                    for j in range(QC):
                        q0 = j * QW
                        es = [
                            epool.tile([128, NKT, QW], BF16, tag="e", name=f"e{hp}_{j}_{hf}")
                            for hf in range(2)
                        ]
                        for kt in range(NKT):
                            for half, e in enumerate(es):
                                p0 = half * 64
                                ps = scps.tile([128, 512], F32, tag="sc",
                                               name=f"sc{hp}_{j}_{kt}_{half}")
                                nc.tensor.matmul(
                                    ps[:, :QW],
                                    kh_p[p0:p0 + 64, kt * 128:(kt + 1) * 128],
                                    qh_s[p0:p0 + 64, hp, q0:q0 + QW],
                                    start=True, stop=True,
                                )
                                nc.scalar.activation(
                                    e[:, kt, :], ps[:, :QW],
                                    mybir.ActivationFunctionType.Exp,
                                    bias=maskb_s[:, kt:kt + 1], scale=SCALE,
                                )
                        for half, e in enumerate(es):
                            av = avps.tile([128, 512], F32, tag="av",
                                           name=f"av{hp}_{j}_{half}")
                            for kt in range(NKT):
                                nc.tensor.matmul(
                                    av[:, :QW],
                                    vh_p[:, kt, half, :],
                                    e[:, kt, :],
                                    start=(kt == 0), stop=(kt == NKT - 1),
                                )
                            rb64 = rpool.tile([128, QW], F32, tag="rb64",
                                              name=f"rb64_{hp}_{j}_{half}")
                            nc.vector.reciprocal(rb64[64:128, :], av[64:128, :QW])
                            rb0 = rpool.tile([64, QW], F32, tag="rb0",
                                             name=f"rb0_{hp}_{j}_{half}")
                            nc.sync.dma_start(out=rb0[:, :], in_=rb64[64:128, :])
                            if half == 0:
                                nc.vector.tensor_mul(
                                    avT_s[0:dh, hp, q0:q0 + QW], av[0:dh, :QW], rb0[:, :]
                                )
                            else:
                                avn = avnpool.tile([dh, QW], BF16, tag="avn",
                                                   name=f"avn{hp}_{j}")
                                nc.vector.tensor_mul(avn[:, :], av[0:dh, :QW], rb0[:, :])
                                nc.sync.dma_start(
                                    out=avT_s[64:128, hp, q0:q0 + QW], in_=avn[:, :]
                                )

            # ---- output projection ----
            wp_s = load_w(wp, "wp_s")
            with (
                tc.tile_pool(name="o_ps", bufs=3, space="PSUM") as ops,
                tc.tile_pool(name="obuf", bufs=3) as obuf,
            ):
                for qt in range(nq // 128):
                    for o0, ow in _chunks(d):
                        ps = ops.tile([128, 512], F32, tag="o", name=f"o{qt}_{o0}")
                        for dc in range(OC):
                            nc.tensor.matmul(
                                ps[:, :ow],
                                avT_s[:, dc, qt * 128:(qt + 1) * 128],
                                wp_s[:, dc, o0:o0 + ow],
                                start=(dc == 0), stop=(dc == OC - 1),
                            )
                        ot = obuf.tile([128, 512], F32, tag="ot", name=f"ot{qt}_{o0}")
                        nc.vector.tensor_add(ot[:, :ow], ps[:, :ow], bp_s[:, o0:o0 + ow])
                        nc.sync.dma_start(
                            out=out[qt * 128:(qt + 1) * 128, o0:o0 + ow], in_=ot[:, :ow]
                        )

    nc.compile()
    return nc


def host_prep(q, k, v, attention_mask, Wq, Wk, Wv, Wp, bp, nq_per_core=None):
    """Pack valid keys, slice + cast full inputs into per-core input maps."""
    nq = nq_per_core or (NQ_FULL * B // N_CORES)
    bsz, nk_full = attention_mask.shape
    cores_per_b = N_CORES // bsz
    idxs = [np.flatnonzero(attention_mask[b]) for b in range(bsz)]
    nk = max(128, -(-max(len(ix) for ix in idxs) // 128) * 128)  # padded packed len
    nkt = nk // 128

    wqT = np.ascontiguousarray(Wq.T).astype(NPBF16)
    wkT = np.ascontiguousarray(Wk.T).astype(NPBF16)
    wvT = np.ascontiguousarray(Wv.T).astype(NPBF16)
    wpT = np.ascontiguousarray(Wp.T).astype(NPBF16)
    bpb = np.ascontiguousarray(bp[None, :]).astype(np.float32)

    packed = []
    for b in range(bsz):
        ix = idxs[b]
        kp = np.zeros((nk, k.shape[2]), NPBF16)
        vp = np.zeros((nk, v.shape[2]), NPBF16)
        kp[:len(ix)] = k[b][ix].astype(NPBF16)
        vp[:len(ix)] = v[b][ix].astype(NPBF16)
        mb = np.full(nk, MASK_NEG, np.float32)
        mb[:len(ix)] = 0.0
        packed.append((kp, vp, np.ascontiguousarray(mb.reshape(nkt, 128).T)))

    in_maps = []
    for c in range(N_CORES):
        b, qi = divmod(c, cores_per_b)
        kp, vp, mb = packed[b]
        in_maps.append({
            "xq": np.ascontiguousarray(q[b, qi * nq:(qi + 1) * nq]).astype(NPBF16),
            "xk": kp, "xv": vp,
            "wq": wqT, "wk": wkT, "wv": wvT, "wp": wpT,
            "maskb": mb, "bpb": bpb,
        })
    return in_maps, nk


_NC_CACHE = {}


def get_nc(nq, nk):
    key = (nq, nk)
    if key not in _NC_CACHE:
        _NC_CACHE[key] = build_nc(nq, nk)
    return _NC_CACHE[key]


def kernel(q, k, v, attention_mask, Wq, Wk, Wv, Wp, bp):
    nq = NQ_FULL * B // N_CORES
    in_maps, nk = host_prep(q, k, v, attention_mask, Wq, Wk, Wv, Wp, bp)
    nc = get_nc(nq, nk)
    res = run_bass_kernel_spmd(nc, in_maps, core_ids=list(range(N_CORES)))
    cores_per_b = N_CORES // B
    out = np.empty((B, NQ_FULL, D), np.float32)
    for c in range(N_CORES):
        b, qi = divmod(c, cores_per_b)
        out[b, qi * nq:(qi + 1) * nq] = res.results[c]["out"]
    return out


# revision 14
# speedup vs baseline: 2.4664x; 1.3312x over previous
"""Bass/Tile Trainium2 kernel for nn_CrossAttention (B=4, Nq=Nk=2048, D=1024, H=16).

Sharding: 8 cores; core c handles batch b=c//2, query rows [(c%2)*1024, (c%2+1)*1024).

Ragged-sequence optimization: valid keys (attention_mask==1) are packed on the host,
so the kernel only attends over ~Nk/2 keys; pad rows get a -60 additive bias before
exp (softmax over packed keys == masked softmax over the full set). Only the last
`nbias` key tiles can contain pad, so all earlier exps skip the bias operand and
fuse two 512-wide score tiles per activation op.

Per-core pipeline (all matmul operands bf16, fp32 PSUM accumulation):
  upfront: vh_all = [v @ Wv | ones-block] for all heads (N=512 matmuls)
  per head-pair hp (projections double-buffered to overlap with attention):
    kh_T pair = Wk-chunk @ k.T ; qh_T pair = Wq-chunk @ q.T
    scores_T[k, q] per 128-k tile; E = exp(scores*SCALE (+ maskbias[k] on tail))
    av[128, q] = vh_aug.T @ E  -> rows 0-63 value, rows 64-127 softmax denominator
    avT = av[0:64] * recip_approx(av[64:128])
  out = avT_all @ Wp.T + bp
"""
import numpy as np
import ml_dtypes

import concourse.bass as bass
import concourse.mybir as mybir
import concourse.tile as tile
from concourse import bacc
from concourse.bass_utils import run_bass_kernel_spmd

F32 = mybir.dt.float32
BF16 = mybir.dt.bfloat16
NPBF16 = ml_dtypes.bfloat16

B, NQ_FULL, NK_FULL, D, H, DH = 4, 2048, 2048, 1024, 16, 64
SCALE = DH ** -0.5
MASK_NEG = -60.0  # additive bias (post-scale) for pad keys; exp(-60) ~ 9e-27
N_CORES = 8


def _chunks(n, w=512):
    out, j = [], 0
    while j < n:
        out.append((j, min(w, n - j)))
        j += min(w, n - j)
    return out


def build_nc(nq, nk, nbias=2, d=D, h=H):
    """Per-core Bass program. nq = q rows/core, nk = packed key rows,
    nbias = # tail key-tiles that may contain pad rows (get the bias operand)."""
    dh = d // h
    assert dh == 64 and d % 128 == 0 and nk % 128 == 0
    IC = d // 128          # contraction chunks for projections
    OC = d // 128          # output chunks (128 rows each)
    NKT = nk // 128        # key tiles
    QC = max(1, nq // 512)  # q chunks of <=512
    QW = min(nq, 512)      # q chunk width
    HP = h // 2            # head pairs
    nbias = min(nbias, NKT)

    nc = bacc.Bacc("TRN2", target_bir_lowering=False, debug=False)

    xq = nc.declare_dram_parameter("xq", [nq, d], BF16, isOutput=False)
    xk = nc.declare_dram_parameter("xk", [nk, d], BF16, isOutput=False)
    xv = nc.declare_dram_parameter("xv", [nk, d], BF16, isOutput=False)
    wq = nc.declare_dram_parameter("wq", [d, d], BF16, isOutput=False)  # Wq.T [in, out]
    wk = nc.declare_dram_parameter("wk", [d, d], BF16, isOutput=False)
    wv = nc.declare_dram_parameter("wv", [d, d], BF16, isOutput=False)
    wp = nc.declare_dram_parameter("wp", [d, d], BF16, isOutput=False)
    maskb = nc.declare_dram_parameter("maskb", [128, NKT], F32, isOutput=False)
    bpb = nc.declare_dram_parameter("bpb", [1, d], F32, isOutput=False)
    out = nc.declare_dram_parameter("out", [nq, d], F32, isOutput=True)

    with tile.TileContext(nc) as tc:
        with (
            tc.tile_pool(name="wpool", bufs=2) as wpool,
            tc.tile_pool(name="const", bufs=1) as cpool,
            tc.tile_pool(name="acts", bufs=1) as apool,
            tc.tile_pool(name="xT", bufs=1) as xpool,
            tc.tile_pool(name="mm_ps", bufs=2, space="PSUM") as mmps,
            tc.tile_pool(name="qkpair", bufs=2) as qkpool,
        ):
            # --- constants ---
            maskb_s = cpool.tile([128, NKT], F32, tag="maskb")
            nc.sync.dma_start(out=maskb_s[:, :], in_=maskb[:, :])
            bp_row = cpool.tile([1, d], F32, tag="bp_row")
            nc.sync.dma_start(out=bp_row[:, :], in_=bpb[:, :])
            bp_s = cpool.tile([128, d], F32, tag="bp")
            nc.gpsimd.partition_broadcast(bp_s[:, :], bp_row[:, :])

            avT_s = apool.tile([128, OC, nq], BF16, tag="avT")   # normalized att out, T
            vh_s = apool.tile([128, NKT, h, 128], BF16, tag="vh")  # [vh | ones] per head

            def load_w(wdram, name):
                w_s = wpool.tile([128, IC, d], BF16, tag="W", name=name)
                nc.sync.dma_start(
                    out=w_s[:, :, :],
                    in_=wdram.ap().rearrange("(c p) o -> p c o", p=128),
                )
                return w_s

            def load_xT(xdram, n, name):
                xT = xpool.tile([128, IC, n], BF16, tag=name, name=name)
                for ic in range(IC):
                    nc.sync.dma_start_transpose(
                        out=xT[:, ic, :], in_=xdram[:, ic * 128:(ic + 1) * 128]
                    )
                return xT

            # ---- V projection upfront (all heads, N=512) ----
            wv_s = load_w(wv, "wv_s")
            xvT = load_xT(xv, nk, "xvT")
            nc.gpsimd.memset(vh_s[:, :, :, dh:], 1.0)
            for kt in range(NKT):
                for half in range(2):
                    ps = mmps.tile([128, 512], F32, tag="ps", name=f"vps{kt}_{half}")
                    for ic in range(IC):
                        nc.tensor.matmul(
                            ps[:, :],
                            xvT[:, ic, kt * 128:(kt + 1) * 128],
                            wv_s[:, ic, half * 512:(half + 1) * 512],
                            start=(ic == 0), stop=(ic == IC - 1),
                        )
                    nc.vector.tensor_copy(
                        vh_s[:, kt, 8 * half:8 * half + 8, 0:dh],
                        ps[:, :].rearrange("p (a b) -> p a b", b=dh),
                    )

            wq_s = load_w(wq, "wq_s")
            wk_s = load_w(wk, "wk_s")
            xqT = load_xT(xq, nq, "xqT")
            xkT = load_xT(xk, nk, "xkT")

            # ---- head-pair loop: K/Q projection + attention ----
            with (
                tc.tile_pool(name="epool", bufs=3) as epool,
                tc.tile_pool(name="sc_ps", bufs=2, space="PSUM") as scps,
                tc.tile_pool(name="av_ps", bufs=2, space="PSUM") as avps,
                tc.tile_pool(name="rpool", bufs=3) as rpool,
                tc.tile_pool(name="avn", bufs=2) as avnpool,
            ):
                for hp in range(HP):
                    # K/Q projections for this pair -> [128, n] (2 heads stacked)
                    kh_p = qkpool.tile([128, nk], BF16, tag="kh", name=f"kh{hp}")
                    for j0, jw in _chunks(nk):
                        ps = mmps.tile([128, 512], F32, tag="ps", name=f"kps{hp}_{j0}")
                        for ic in range(IC):
                            nc.tensor.matmul(
                                ps[:, :jw],
                                wk_s[:, ic, hp * 128:(hp + 1) * 128],
                                xkT[:, ic, j0:j0 + jw],
                                start=(ic == 0), stop=(ic == IC - 1),
                            )
                        nc.vector.tensor_copy(kh_p[:, j0:j0 + jw], ps[:, :jw])
                    qh_p = qkpool.tile([128, nq], BF16, tag="qh", name=f"qh{hp}")
                    for j0, jw in _chunks(nq):
                        ps = mmps.tile([128, 512], F32, tag="ps", name=f"qps{hp}_{j0}")
                        for ic in range(IC):
                            nc.tensor.matmul(
                                ps[:, :jw],
                                wq_s[:, ic, hp * 128:(hp + 1) * 128],
                                xqT[:, ic, j0:j0 + jw],
                                start=(ic == 0), stop=(ic == IC - 1),
                            )
                        nc.vector.tensor_copy(qh_p[:, j0:j0 + jw], ps[:, :jw])

                    # attention for both heads of the pair
                    for j in range(QC):
                        q0 = j * QW
                        es = [
                            epool.tile([128, NKT, QW], BF16, tag="e", name=f"e{hp}_{j}_{hf}")
                            for hf in range(2)
                        ]
                        for kp in range((NKT + 1) // 2):
                            kts = [kt for kt in (2 * kp, 2 * kp + 1) if kt < NKT]
                            for half, e in enumerate(es):
                                p0 = half * 64
                                ps = scps.tile([128, 2, 512], F32, tag="sc",
                                               name=f"sc{hp}_{j}_{kp}_{half}")
                                for si, kt in enumerate(kts):
                                    nc.tensor.matmul(
                                        ps[:, si, :QW],
                                        kh_p[p0:p0 + 64, kt * 128:(kt + 1) * 128],
                                        qh_p[p0:p0 + 64, q0:q0 + QW],
                                        start=True, stop=True,
                                    )
                                if kts[-1] < NKT - nbias and len(kts) == 2:
                                    nc.scalar.activation(
                                        e[:, kts[0]:kts[0] + 2, :], ps[:, :, :QW],
                                        mybir.ActivationFunctionType.Exp,
                                        bias=0.0, scale=SCALE,
                                    )
                                else:
                                    for si, kt in enumerate(kts):
                                        if kt >= NKT - nbias:
                                            nc.scalar.activation(
                                                e[:, kt, :], ps[:, si, :QW],
                                                mybir.ActivationFunctionType.Exp,
                                                bias=maskb_s[:, kt:kt + 1], scale=SCALE,
                                            )
                                        else:
                                            nc.scalar.activation(
                                                e[:, kt, :], ps[:, si, :QW],
                                                mybir.ActivationFunctionType.Exp,
                                                bias=0.0, scale=SCALE,
                                            )
                        for half, e in enumerate(es):
                            hh = 2 * hp + half
                            av = avps.tile([128, 512], F32, tag="av",
                                           name=f"av{hp}_{j}_{half}")
                            for kt in range(NKT):
                                nc.tensor.matmul(
                                    av[:, :QW],
                                    vh_s[:, kt, hh, :],
                                    e[:, kt, :],
                                    start=(kt == 0), stop=(kt == NKT - 1),
                                )
                            d64 = rpool.tile([128, QW], F32, tag="d64",
                                             name=f"d64_{hp}_{j}_{half}")
                            nc.vector.tensor_copy(d64[64:128, :], av[64:128, :QW])
                            d0 = rpool.tile([64, QW], F32, tag="d0",
                                            name=f"d0_{hp}_{j}_{half}")
                            nc.sync.dma_start(out=d0[:, :], in_=d64[64:128, :])
                            rb0 = rpool.tile([64, QW], F32, tag="rb0",
                                             name=f"rb0_{hp}_{j}_{half}")
                            nc.vector.reciprocal_approx_fast(out=rb0[:, :], in_=d0[:, :])
                            if half == 0:
                                nc.vector.tensor_mul(
                                    avT_s[0:dh, hp, q0:q0 + QW], av[0:dh, :QW], rb0[:, :]
                                )
                            else:
                                avn = avnpool.tile([dh, QW], BF16, tag="avn",
                                                   name=f"avn{hp}_{j}")
                                nc.vector.tensor_mul(avn[:, :], av[0:dh, :QW], rb0[:, :])
                                nc.sync.dma_start(
                                    out=avT_s[64:128, hp, q0:q0 + QW], in_=avn[:, :]
                                )

            # ---- output projection ----
            wp_s = load_w(wp, "wp_s")
            with (
                tc.tile_pool(name="o_ps", bufs=3, space="PSUM") as ops,
                tc.tile_pool(name="obuf", bufs=3) as obuf,
            ):
                for qt in range(nq // 128):
                    for o0, ow in _chunks(d):
                        ps = ops.tile([128, 512], F32, tag="o", name=f"o{qt}_{o0}")
                        for dc in range(OC):
                            nc.tensor.matmul(
                                ps[:, :ow],
                                avT_s[:, dc, qt * 128:(qt + 1) * 128],
                                wp_s[:, dc, o0:o0 + ow],
                                start=(dc == 0), stop=(dc == OC - 1),
                            )
                        ot = obuf.tile([128, 512], F32, tag="ot", name=f"ot{qt}_{o0}")
                        nc.vector.tensor_add(ot[:, :ow], ps[:, :ow], bp_s[:, o0:o0 + ow])
                        nc.sync.dma_start(
                            out=out[qt * 128:(qt + 1) * 128, o0:o0 + ow], in_=ot[:, :ow]
                        )

    nc.compile()
    return nc


def host_prep(q, k, v, attention_mask, Wq, Wk, Wv, Wp, bp, nq_per_core=None):
    """Pack valid keys, slice + cast full inputs into per-core input maps."""
    nq = nq_per_core or (NQ_FULL * B // N_CORES)
    bsz, nk_full = attention_mask.shape
    cores_per_b = N_CORES // bsz
    idxs = [np.flatnonzero(attention_mask[b]) for b in range(bsz)]
    nv_min = min(len(ix) for ix in idxs)
    nk = max(128, -(-max(len(ix) for ix in idxs) // 128) * 128)  # padded packed len
    nkt = nk // 128
    nbias = max(1, -(-(nk - nv_min) // 128))

    wqT = np.ascontiguousarray(Wq.T).astype(NPBF16)
    wkT = np.ascontiguousarray(Wk.T).astype(NPBF16)
    wvT = np.ascontiguousarray(Wv.T).astype(NPBF16)
    wpT = np.ascontiguousarray(Wp.T).astype(NPBF16)
    bpb = np.ascontiguousarray(bp[None, :]).astype(np.float32)

    packed = []
    for b in range(bsz):
        ix = idxs[b]
        kp = np.zeros((nk, k.shape[2]), NPBF16)
        vp = np.zeros((nk, v.shape[2]), NPBF16)
        kp[:len(ix)] = k[b][ix].astype(NPBF16)
        vp[:len(ix)] = v[b][ix].astype(NPBF16)
        mb = np.full(nk, MASK_NEG, np.float32)
        mb[:len(ix)] = 0.0
        packed.append((kp, vp, np.ascontiguousarray(mb.reshape(nkt, 128).T)))

    in_maps = []
    for c in range(N_CORES):
        b, qi = divmod(c, cores_per_b)
        kp, vp, mb = packed[b]
        in_maps.append({
            "xq": np.ascontiguousarray(q[b, qi * nq:(qi + 1) * nq]).astype(NPBF16),
            "xk": kp, "xv": vp,
            "wq": wqT, "wk": wkT, "wv": wvT, "wp": wpT,
            "maskb": mb, "bpb": bpb,
        })
    return in_maps, nk, nbias


_NC_CACHE = {}


def get_nc(nq, nk, nbias=2):
    key = (nq, nk, nbias)
    if key not in _NC_CACHE:
        _NC_CACHE[key] = build_nc(nq, nk, nbias)
    return _NC_CACHE[key]


def kernel(q, k, v, attention_mask, Wq, Wk, Wv, Wp, bp):
    nq = NQ_FULL * B // N_CORES
    in_maps, nk, nbias = host_prep(q, k, v, attention_mask, Wq, Wk, Wv, Wp, bp)
    nc = get_nc(nq, nk, nbias)
    res = run_bass_kernel_spmd(nc, in_maps, core_ids=list(range(N_CORES)))
    cores_per_b = N_CORES // B
    out = np.empty((B, NQ_FULL, D), np.float32)
    for c in range(N_CORES):
        b, qi = divmod(c, cores_per_b)
        out[b, qi * nq:(qi + 1) * nq] = res.results[c]["out"]
    return out


# revision 15
# speedup vs baseline: 2.5369x; 1.0286x over previous
"""Bass/Tile Trainium2 kernel for nn_CrossAttention (B=4, Nq=Nk=2048, D=1024, H=16).

Sharding: 8 cores; core c handles batch b=c//2, query rows [(c%2)*1024, (c%2+1)*1024).

Ragged-sequence optimization: valid keys (attention_mask==1) are packed on the host,
so the kernel only attends over ~Nk/2 keys; pad rows get a -60 additive bias before
exp (softmax over packed keys == masked softmax over the full set). Only the last
`nbias` key tiles can contain pad, so all earlier exps skip the bias operand and
fuse two 512-wide score tiles per activation op.

Per-core pipeline (all matmul operands bf16, fp32 PSUM accumulation):
  upfront: vh_all = [v @ Wv | ones-block] for all heads (N=512 matmuls)
  per head-pair hp (projections double-buffered to overlap with attention):
    kh_T pair = Wk-chunk @ k.T ; qh_T pair = Wq-chunk @ q.T
    scores_T[k, q] per 128-k tile; E = exp(scores*SCALE (+ maskbias[k] on tail))
    av[128, q] = vh_aug.T @ E  -> rows 0-63 value, rows 64-127 softmax denominator
    avT = av[0:64] * recip_approx(av[64:128])
  out = avT_all @ Wp.T + bp
"""
import numpy as np
import ml_dtypes

import concourse.bass as bass
import concourse.mybir as mybir
import concourse.tile as tile
from concourse import bacc
from concourse.bass_utils import run_bass_kernel_spmd

F32 = mybir.dt.float32
BF16 = mybir.dt.bfloat16
NPBF16 = ml_dtypes.bfloat16

B, NQ_FULL, NK_FULL, D, H, DH = 4, 2048, 2048, 1024, 16, 64
SCALE = DH ** -0.5
MASK_NEG = -60.0  # additive bias (post-scale) for pad keys; exp(-60) ~ 9e-27
N_CORES = 8


def _chunks(n, w=512):
    out, j = [], 0
    while j < n:
        out.append((j, min(w, n - j)))
        j += min(w, n - j)
    return out


def build_nc(nq, nk, nbias=2, d=D, h=H):
    """Per-core Bass program. nq = q rows/core, nk = packed key rows,
    nbias = # tail key-tiles that may contain pad rows (get the bias operand)."""
    dh = d // h
    assert dh == 64 and d % 128 == 0 and nk % 128 == 0
    IC = d // 128          # contraction chunks for projections
    OC = d // 128          # output chunks (128 rows each)
    NKT = nk // 128        # key tiles
    QC = max(1, nq // 512)  # q chunks of <=512
    QW = min(nq, 512)      # q chunk width
    HP = h // 2            # head pairs
    nbias = min(nbias, NKT)

    nc = bacc.Bacc("TRN2", target_bir_lowering=False, debug=False)

    xq = nc.declare_dram_parameter("xq", [nq, d], BF16, isOutput=False)
    xk = nc.declare_dram_parameter("xk", [nk, d], BF16, isOutput=False)
    xv = nc.declare_dram_parameter("xv", [nk, d], BF16, isOutput=False)
    wq = nc.declare_dram_parameter("wq", [d, d], BF16, isOutput=False)  # Wq.T [in, out]
    wk = nc.declare_dram_parameter("wk", [d, d], BF16, isOutput=False)
    wv = nc.declare_dram_parameter("wv", [d, d], BF16, isOutput=False)
    wp = nc.declare_dram_parameter("wp", [d, d], BF16, isOutput=False)
    maskb = nc.declare_dram_parameter("maskb", [128, NKT], F32, isOutput=False)
    bpb = nc.declare_dram_parameter("bpb", [1, d], F32, isOutput=False)
    out = nc.declare_dram_parameter("out", [nq, d], F32, isOutput=True)

    with tile.TileContext(nc) as tc:
        with (
            tc.tile_pool(name="wpool", bufs=2) as wpool,
            tc.tile_pool(name="const", bufs=1) as cpool,
            tc.tile_pool(name="acts", bufs=1) as apool,
            tc.tile_pool(name="xT", bufs=1) as xpool,
            tc.tile_pool(name="mm_ps", bufs=2, space="PSUM") as mmps,
            tc.tile_pool(name="qkpair", bufs=2) as qkpool,
        ):
            # --- constants ---
            maskb_s = cpool.tile([128, NKT], F32, tag="maskb")
            nc.sync.dma_start(out=maskb_s[:, :], in_=maskb[:, :])
            bp_row = cpool.tile([1, d], F32, tag="bp_row")
            nc.sync.dma_start(out=bp_row[:, :], in_=bpb[:, :])
            bp_s = cpool.tile([128, d], F32, tag="bp")
            nc.gpsimd.partition_broadcast(bp_s[:, :], bp_row[:, :])

            avT_s = apool.tile([128, OC, nq], BF16, tag="avT")   # normalized att out, T
            vh_s = apool.tile([128, NKT, h, 128], BF16, tag="vh")  # [vh | ones] per head

            def load_w(wdram, name):
                w_s = wpool.tile([128, IC, d], BF16, tag="W", name=name)
                nc.sync.dma_start(
                    out=w_s[:, :, :],
                    in_=wdram.ap().rearrange("(c p) o -> p c o", p=128),
                )
                return w_s

            def load_xT(xdram, n, name):
                xT = xpool.tile([128, IC, n], BF16, tag=name, name=name)
                for ic in range(IC):
                    nc.sync.dma_start_transpose(
                        out=xT[:, ic, :], in_=xdram[:, ic * 128:(ic + 1) * 128]
                    )
                return xT

            # ---- V projection upfront (all heads, N=512) ----
            wv_s = load_w(wv, "wv_s")
            xvT = load_xT(xv, nk, "xvT")
            nc.gpsimd.memset(vh_s[:, :, :, dh:], 1.0)
            for kt in range(NKT):
                for half in range(2):
                    ps = mmps.tile([128, 512], F32, tag="ps", name=f"vps{kt}_{half}")
                    for ic in range(IC):
                        nc.tensor.matmul(
                            ps[:, :],
                            xvT[:, ic, kt * 128:(kt + 1) * 128],
                            wv_s[:, ic, half * 512:(half + 1) * 512],
                            start=(ic == 0), stop=(ic == IC - 1),
                        )
                    nc.vector.tensor_copy(
                        vh_s[:, kt, 8 * half:8 * half + 8, 0:dh],
                        ps[:, :].rearrange("p (a b) -> p a b", b=dh),
                    )

            wq_s = load_w(wq, "wq_s")
            wk_s = load_w(wk, "wk_s")
            xqT = load_xT(xq, nq, "xqT")
            xkT = load_xT(xk, nk, "xkT")

            # ---- head-pair loop: K/Q projection + attention ----
            with (
                tc.tile_pool(name="epool", bufs=3) as epool,
                tc.tile_pool(name="sc_ps", bufs=2, space="PSUM") as scps,
                tc.tile_pool(name="av_ps", bufs=2, space="PSUM") as avps,
                tc.tile_pool(name="rpool", bufs=3) as rpool,
                tc.tile_pool(name="avn", bufs=2) as avnpool,
            ):
                for hp in range(HP):
                    # K/Q projections for this pair -> [128, n] (2 heads stacked)
                    kh_p = qkpool.tile([128, nk], BF16, tag="kh", name=f"kh{hp}")
                    for j0, jw in _chunks(nk):
                        ps = mmps.tile([128, 512], F32, tag="ps", name=f"kps{hp}_{j0}")
                        for ic in range(IC):
                            nc.tensor.matmul(
                                ps[:, :jw],
                                wk_s[:, ic, hp * 128:(hp + 1) * 128],
                                xkT[:, ic, j0:j0 + jw],
                                start=(ic == 0), stop=(ic == IC - 1),
                            )
                        nc.vector.tensor_copy(kh_p[:, j0:j0 + jw], ps[:, :jw])
                    qh_p = qkpool.tile([128, nq], BF16, tag="qh", name=f"qh{hp}")
                    for j0, jw in _chunks(nq):
                        ps = mmps.tile([128, 512], F32, tag="ps", name=f"qps{hp}_{j0}")
                        for ic in range(IC):
                            nc.tensor.matmul(
                                ps[:, :jw],
                                wq_s[:, ic, hp * 128:(hp + 1) * 128],
                                xqT[:, ic, j0:j0 + jw],
                                start=(ic == 0), stop=(ic == IC - 1),
                            )
                        nc.vector.tensor_copy(qh_p[:, j0:j0 + jw], ps[:, :jw])

                    # attention for both heads of the pair
                    for j in range(QC):
                        q0 = j * QW
                        es = [
                            epool.tile([128, NKT, QW], BF16, tag="e", name=f"e{hp}_{j}_{hf}")
                            for hf in range(2)
                        ]
                        for kp in range((NKT + 1) // 2):
                            kts = [kt for kt in (2 * kp, 2 * kp + 1) if kt < NKT]
                            for half, e in enumerate(es):
                                p0 = half * 64
                                ps = scps.tile([128, 2, 512], F32, tag="sc",
                                               name=f"sc{hp}_{j}_{kp}_{half}")
                                for si, kt in enumerate(kts):
                                    nc.tensor.matmul(
                                        ps[:, si, :QW],
                                        kh_p[p0:p0 + 64, kt * 128:(kt + 1) * 128],
                                        qh_p[p0:p0 + 64, q0:q0 + QW],
                                        start=True, stop=True,
                                        tile_position=(p0, 0),
                                    )
                                if kts[-1] < NKT - nbias and len(kts) == 2:
                                    nc.scalar.activation(
                                        e[:, kts[0]:kts[0] + 2, :], ps[:, :, :QW],
                                        mybir.ActivationFunctionType.Exp,
                                        bias=0.0, scale=SCALE,
                                    )
                                else:
                                    for si, kt in enumerate(kts):
                                        if kt >= NKT - nbias:
                                            nc.scalar.activation(
                                                e[:, kt, :], ps[:, si, :QW],
                                                mybir.ActivationFunctionType.Exp,
                                                bias=maskb_s[:, kt:kt + 1], scale=SCALE,
                                            )
                                        else:
                                            nc.scalar.activation(
                                                e[:, kt, :], ps[:, si, :QW],
                                                mybir.ActivationFunctionType.Exp,
                                                bias=0.0, scale=SCALE,
                                            )
                        for half, e in enumerate(es):
                            hh = 2 * hp + half
                            av = avps.tile([128, 512], F32, tag="av",
                                           name=f"av{hp}_{j}_{half}")
                            for kt in range(NKT):
                                nc.tensor.matmul(
                                    av[:, :QW],
                                    vh_s[:, kt, hh, :],
                                    e[:, kt, :],
                                    start=(kt == 0), stop=(kt == NKT - 1),
                                )
                            d64 = rpool.tile([128, QW], F32, tag="d64",
                                             name=f"d64_{hp}_{j}_{half}")
                            nc.vector.tensor_copy(d64[64:128, :], av[64:128, :QW])
                            d0 = rpool.tile([64, QW], F32, tag="d0",
                                            name=f"d0_{hp}_{j}_{half}")
                            nc.sync.dma_start(out=d0[:, :], in_=d64[64:128, :])
                            rb0 = rpool.tile([64, QW], F32, tag="rb0",
                                             name=f"rb0_{hp}_{j}_{half}")
                            nc.vector.reciprocal_approx_fast(out=rb0[:, :], in_=d0[:, :])
                            if half == 0:
                                nc.vector.tensor_mul(
                                    avT_s[0:dh, hp, q0:q0 + QW], av[0:dh, :QW], rb0[:, :]
                                )
                            else:
                                avn = avnpool.tile([dh, QW], BF16, tag="avn",
                                                   name=f"avn{hp}_{j}")
                                nc.vector.tensor_mul(avn[:, :], av[0:dh, :QW], rb0[:, :])
                                nc.sync.dma_start(
                                    out=avT_s[64:128, hp, q0:q0 + QW], in_=avn[:, :]
                                )

            # ---- output projection ----
            wp_s = load_w(wp, "wp_s")
            with (
                tc.tile_pool(name="o_ps", bufs=3, space="PSUM") as ops,
                tc.tile_pool(name="obuf", bufs=3) as obuf,
            ):
                for qt in range(nq // 128):
                    for o0, ow in _chunks(d):
                        ps = ops.tile([128, 512], F32, tag="o", name=f"o{qt}_{o0}")
                        for dc in range(OC):
                            nc.tensor.matmul(
                                ps[:, :ow],
                                avT_s[:, dc, qt * 128:(qt + 1) * 128],
                                wp_s[:, dc, o0:o0 + ow],
                                start=(dc == 0), stop=(dc == OC - 1),
                            )
                        ot = obuf.tile([128, 512], F32, tag="ot", name=f"ot{qt}_{o0}")
                        nc.vector.tensor_add(ot[:, :ow], ps[:, :ow], bp_s[:, o0:o0 + ow])
                        nc.sync.dma_start(
                            out=out[qt * 128:(qt + 1) * 128, o0:o0 + ow], in_=ot[:, :ow]
                        )

    nc.compile()
    return nc


def host_prep(q, k, v, attention_mask, Wq, Wk, Wv, Wp, bp, nq_per_core=None):
    """Pack valid keys, slice + cast full inputs into per-core input maps."""
    nq = nq_per_core or (NQ_FULL * B // N_CORES)
    bsz, nk_full = attention_mask.shape
    cores_per_b = N_CORES // bsz
    idxs = [np.flatnonzero(attention_mask[b]) for b in range(bsz)]
    nv_min = min(len(ix) for ix in idxs)
    nk = max(128, -(-max(len(ix) for ix in idxs) // 128) * 128)  # padded packed len
    nkt = nk // 128
    nbias = max(1, -(-(nk - nv_min) // 128))

    wqT = np.ascontiguousarray(Wq.T).astype(NPBF16)
    wkT = np.ascontiguousarray(Wk.T).astype(NPBF16)
    wvT = np.ascontiguousarray(Wv.T).astype(NPBF16)
    wpT = np.ascontiguousarray(Wp.T).astype(NPBF16)
    bpb = np.ascontiguousarray(bp[None, :]).astype(np.float32)

    packed = []
    for b in range(bsz):
        ix = idxs[b]
        kp = np.zeros((nk, k.shape[2]), NPBF16)
        vp = np.zeros((nk, v.shape[2]), NPBF16)
        kp[:len(ix)] = k[b][ix].astype(NPBF16)
        vp[:len(ix)] = v[b][ix].astype(NPBF16)
        mb = np.full(nk, MASK_NEG, np.float32)
        mb[:len(ix)] = 0.0
        packed.append((kp, vp, np.ascontiguousarray(mb.reshape(nkt, 128).T)))

    in_maps = []
    for c in range(N_CORES):
        b, qi = divmod(c, cores_per_b)
        kp, vp, mb = packed[b]
        in_maps.append({
            "xq": np.ascontiguousarray(q[b, qi * nq:(qi + 1) * nq]).astype(NPBF16),
            "xk": kp, "xv": vp,
            "wq": wqT, "wk": wkT, "wv": wvT, "wp": wpT,
            "maskb": mb, "bpb": bpb,
        })
    return in_maps, nk, nbias


_NC_CACHE = {}


def get_nc(nq, nk, nbias=2):
    key = (nq, nk, nbias)
    if key not in _NC_CACHE:
        _NC_CACHE[key] = build_nc(nq, nk, nbias)
    return _NC_CACHE[key]


def kernel(q, k, v, attention_mask, Wq, Wk, Wv, Wp, bp):
    nq = NQ_FULL * B // N_CORES
    in_maps, nk, nbias = host_prep(q, k, v, attention_mask, Wq, Wk, Wv, Wp, bp)
    nc = get_nc(nq, nk, nbias)
    res = run_bass_kernel_spmd(nc, in_maps, core_ids=list(range(N_CORES)))
    cores_per_b = N_CORES // B
    out = np.empty((B, NQ_FULL, D), np.float32)
    for c in range(N_CORES):
        b, qi = divmod(c, cores_per_b)
        out[b, qi * nq:(qi + 1) * nq] = res.results[c]["out"]
    return out


# revision 20
# speedup vs baseline: 2.6158x; 1.0311x over previous
"""Bass/Tile Trainium2 kernel for nn_CrossAttention (B=4, Nq=Nk=2048, D=1024, H=16).

Sharding: 8 cores; core c handles batch b=c//2, query rows [(c%2)*1024, (c%2+1)*1024).

Ragged-sequence optimization: valid keys (attention_mask==1) are packed on the host,
so the kernel only attends over ~Nk/2 keys; pad rows get a -60 additive bias before
exp (softmax over packed keys == masked softmax over the full set). Only the last
`nbias` key tiles can contain pad, so all earlier exps skip the bias operand and
fuse two 512-wide score tiles per activation op.

Per-core pipeline (all matmul operands bf16, fp32 PSUM accumulation):
  upfront: vh_all = [v @ Wv | ones-block] for all heads (N=512 matmuls)
  per head-pair hp (projections double-buffered to overlap with attention):
    kh_T pair = Wk-chunk @ k.T ; qh_T pair = Wq-chunk @ q.T
    scores_T[k, q] per 128-k tile; E = exp(scores*SCALE (+ maskbias[k] on tail))
    av[128, q] = vh_aug.T @ E  -> rows 0-63 value, rows 64-127 softmax denominator
    avT = av[0:64] * recip_approx(av[64:128])
  out = avT_all @ Wp.T + bp
"""
import numpy as np
import ml_dtypes

import concourse.bass as bass
import concourse.mybir as mybir
import concourse.tile as tile
from concourse import bacc
from concourse.bass_utils import run_bass_kernel_spmd

F32 = mybir.dt.float32
BF16 = mybir.dt.bfloat16
NPBF16 = ml_dtypes.bfloat16

B, NQ_FULL, NK_FULL, D, H, DH = 4, 2048, 2048, 1024, 16, 64
SCALE = DH ** -0.5
MASK_NEG = -60.0  # additive bias (post-scale) for pad keys; exp(-60) ~ 9e-27
N_CORES = 8


def _chunks(n, w=512):
    out, j = [], 0
    while j < n:
        out.append((j, min(w, n - j)))
        j += min(w, n - j)
    return out


def build_nc(nq, nk, nbias=2, d=D, h=H):
    """Per-core Bass program. nq = q rows/core, nk = packed key rows,
    nbias = # tail key-tiles that may contain pad rows (get the bias operand)."""
    dh = d // h
    assert dh == 64 and d % 128 == 0 and nk % 128 == 0
    IC = d // 128          # contraction chunks for projections
    OC = d // 128          # output chunks (128 rows each)
    NKT = nk // 128        # key tiles
    QC = max(1, nq // 512)  # q chunks of <=512
    QW = min(nq, 512)      # q chunk width
    HP = h // 2            # head pairs
    nbias = min(nbias, NKT)

    nc = bacc.Bacc("TRN2", target_bir_lowering=False, debug=False)

    # activations arrive pre-transposed from the host: [d, n] layout
    xq = nc.declare_dram_parameter("xq", [d, nq], BF16, isOutput=False)
    xk = nc.declare_dram_parameter("xk", [d, nk], BF16, isOutput=False)
    xv = nc.declare_dram_parameter("xv", [d, nk], BF16, isOutput=False)
    wq = nc.declare_dram_parameter("wq", [d, d], BF16, isOutput=False)  # Wq.T [in, out]
    wk = nc.declare_dram_parameter("wk", [d, d], BF16, isOutput=False)
    wv = nc.declare_dram_parameter("wv", [d, d], BF16, isOutput=False)
    wp = nc.declare_dram_parameter("wp", [d, d], BF16, isOutput=False)
    maskb = nc.declare_dram_parameter("maskb", [128, NKT], F32, isOutput=False)
    bpb = nc.declare_dram_parameter("bpb", [1, d], F32, isOutput=False)
    out = nc.declare_dram_parameter("out", [nq, d], F32, isOutput=True)

    with tile.TileContext(nc) as tc:
        with (
            tc.tile_pool(name="wpool", bufs=2) as wpool,
            tc.tile_pool(name="const", bufs=1) as cpool,
            tc.tile_pool(name="acts", bufs=1) as apool,
            tc.tile_pool(name="xT", bufs=1) as xpool,
            tc.tile_pool(name="mm_ps", bufs=2, space="PSUM") as mmps,
            tc.tile_pool(name="qkpair", bufs=2) as qkpool,
        ):
            # --- constants ---
            maskb_s = cpool.tile([128, NKT], F32, tag="maskb")
            nc.sync.dma_start(out=maskb_s[:, :], in_=maskb[:, :])
            bp_row = cpool.tile([1, d], F32, tag="bp_row")
            nc.sync.dma_start(out=bp_row[:, :], in_=bpb[:, :])
            bp_s = cpool.tile([128, d], F32, tag="bp")
            nc.gpsimd.partition_broadcast(bp_s[:, :], bp_row[:, :])

            avT_s = apool.tile([128, OC, nq], BF16, tag="avT")   # normalized att out, T
            vh_s = apool.tile([128, NKT, h, 128], BF16, tag="vh")  # [vh | ones] per head

            def load_w(wdram, name):
                w_s = wpool.tile([128, IC, d], BF16, tag="W", name=name)
                nc.sync.dma_start(
                    out=w_s[:, :, :],
                    in_=wdram.ap().rearrange("(c p) o -> p c o", p=128),
                )
                return w_s

            def load_xT(xdram, n, name):
                xT = xpool.tile([128, IC, n], BF16, tag=name, name=name)
                nc.sync.dma_start(
                    out=xT[:, :, :],
                    in_=xdram.ap().rearrange("(c p) o -> p c o", p=128),
                )
                return xT

            # ---- V projection upfront (all heads, N=512) ----
            wk_s = load_w(wk, "wk_s")
            wv_s = load_w(wv, "wv_s")
            xvT = load_xT(xv, nk, "xvT")
            xqT = load_xT(xq, nq, "xqT")
            xkT = load_xT(xk, nk, "xkT")
            nc.gpsimd.memset(vh_s[:, :, :, dh:], 1.0)
            for kt in range(NKT):
                for half in range(2):
                    ps = mmps.tile([128, 512], F32, tag="ps", name=f"vps{kt}_{half}")
                    for ic in range(IC):
                        nc.tensor.matmul(
                            ps[:, :],
                            xvT[:, ic, kt * 128:(kt + 1) * 128],
                            wv_s[:, ic, half * 512:(half + 1) * 512],
                            start=(ic == 0), stop=(ic == IC - 1),
                        )
                    nc.vector.tensor_copy(
                        vh_s[:, kt, 8 * half:8 * half + 8, 0:dh],
                        ps[:, :].rearrange("p (a b) -> p a b", b=dh),
                    )

            wq_s = load_w(wq, "wq_s")  # takes wv's slot once V-projection drains

            # ---- head-pair loop: K/Q projection + attention ----
            with (
                tc.tile_pool(name="epool", bufs=3) as epool,
                tc.tile_pool(name="sc_ps", bufs=2, space="PSUM") as scps,
                tc.tile_pool(name="av_ps", bufs=2, space="PSUM") as avps,
                tc.tile_pool(name="rpool", bufs=3) as rpool,
                tc.tile_pool(name="avn", bufs=2) as avnpool,
            ):
                for hp in range(HP):
                    # K/Q projections for this pair -> [128, n] (2 heads stacked)
                    kh_p = qkpool.tile([128, nk], BF16, tag="kh", name=f"kh{hp}")
                    for j0, jw in _chunks(nk):
                        ps = mmps.tile([128, 512], F32, tag="ps", name=f"kps{hp}_{j0}")
                        for ic in range(IC):
                            nc.tensor.matmul(
                                ps[:, :jw],
                                wk_s[:, ic, hp * 128:(hp + 1) * 128],
                                xkT[:, ic, j0:j0 + jw],
                                start=(ic == 0), stop=(ic == IC - 1),
                            )
                        nc.vector.tensor_copy(kh_p[:, j0:j0 + jw], ps[:, :jw])
                    qh_p = qkpool.tile([128, nq], BF16, tag="qh", name=f"qh{hp}")
                    for j0, jw in _chunks(nq):
                        ps = mmps.tile([128, 512], F32, tag="ps", name=f"qps{hp}_{j0}")
                        for ic in range(IC):
                            nc.tensor.matmul(
                                ps[:, :jw],
                                wq_s[:, ic, hp * 128:(hp + 1) * 128],
                                xqT[:, ic, j0:j0 + jw],
                                start=(ic == 0), stop=(ic == IC - 1),
                            )
                        nc.vector.tensor_copy(qh_p[:, j0:j0 + jw], ps[:, :jw])

                    # attention for both heads of the pair
                    for j in range(QC):
                        q0 = j * QW
                        es = [
                            epool.tile([128, NKT, QW], BF16, tag="e", name=f"e{hp}_{j}_{hf}")
                            for hf in range(2)
                        ]
                        for kp in range((NKT + 1) // 2):
                            kts = [kt for kt in (2 * kp, 2 * kp + 1) if kt < NKT]
                            for half, e in enumerate(es):
                                p0 = half * 64
                                ps = scps.tile([128, 2, 512], F32, tag="sc",
                                               name=f"sc{hp}_{j}_{kp}_{half}")
                                for si, kt in enumerate(kts):
                                    nc.tensor.matmul(
                                        ps[:, si, :QW],
                                        kh_p[p0:p0 + 64, kt * 128:(kt + 1) * 128],
                                        qh_p[p0:p0 + 64, q0:q0 + QW],
                                        start=True, stop=True,
                                        tile_position=(p0, 0),
                                    )
                                if kts[-1] < NKT - nbias and len(kts) == 2:
                                    nc.scalar.activation(
                                        e[:, kts[0]:kts[0] + 2, :], ps[:, :, :QW],
                                        mybir.ActivationFunctionType.Exp,
                                        bias=0.0, scale=SCALE,
                                    )
                                else:
                                    for si, kt in enumerate(kts):
                                        if kt >= NKT - nbias:
                                            nc.scalar.activation(
                                                e[:, kt, :], ps[:, si, :QW],
                                                mybir.ActivationFunctionType.Exp,
                                                bias=maskb_s[:, kt:kt + 1], scale=SCALE,
                                            )
                                        else:
                                            nc.scalar.activation(
                                                e[:, kt, :], ps[:, si, :QW],
                                                mybir.ActivationFunctionType.Exp,
                                                bias=0.0, scale=SCALE,
                                            )
                        for half, e in enumerate(es):
                            hh = 2 * hp + half
                            av = avps.tile([128, 512], F32, tag="av",
                                           name=f"av{hp}_{j}_{half}")
                            for kt in range(NKT):
                                nc.tensor.matmul(
                                    av[:, :QW],
                                    vh_s[:, kt, hh, :],
                                    e[:, kt, :],
                                    start=(kt == 0), stop=(kt == NKT - 1),
                                )
                            d64 = rpool.tile([128, QW], F32, tag="d64",
                                             name=f"d64_{hp}_{j}_{half}")
                            nc.vector.tensor_copy(d64[64:128, :], av[64:128, :QW])
                            d0 = rpool.tile([64, QW], F32, tag="d0",
                                            name=f"d0_{hp}_{j}_{half}")
                            nc.sync.dma_start(out=d0[:, :], in_=d64[64:128, :])
                            rb0 = rpool.tile([64, QW], F32, tag="rb0",
                                             name=f"rb0_{hp}_{j}_{half}")
                            nc.vector.reciprocal_approx_fast(out=rb0[:, :], in_=d0[:, :])
                            if half == 0:
                                nc.vector.tensor_mul(
                                    avT_s[0:dh, hp, q0:q0 + QW], av[0:dh, :QW], rb0[:, :]
                                )
                            else:
                                avn = avnpool.tile([dh, QW], BF16, tag="avn",
                                                   name=f"avn{hp}_{j}")
                                nc.vector.tensor_mul(avn[:, :], av[0:dh, :QW], rb0[:, :])
                                nc.sync.dma_start(
                                    out=avT_s[64:128, hp, q0:q0 + QW], in_=avn[:, :]
                                )

            # ---- output projection ----
            wp_s = load_w(wp, "wp_s")
            with (
                tc.tile_pool(name="o_ps", bufs=3, space="PSUM") as ops,
                tc.tile_pool(name="obuf", bufs=3) as obuf,
            ):
                for qt in range(nq // 128):
                    for o0, ow in _chunks(d):
                        ps = ops.tile([128, 512], F32, tag="o", name=f"o{qt}_{o0}")
                        for dc in range(OC):
                            nc.tensor.matmul(
                                ps[:, :ow],
                                avT_s[:, dc, qt * 128:(qt + 1) * 128],
                                wp_s[:, dc, o0:o0 + ow],
                                start=(dc == 0), stop=(dc == OC - 1),
                            )
                        ot = obuf.tile([128, 512], F32, tag="ot", name=f"ot{qt}_{o0}")
                        nc.vector.tensor_add(ot[:, :ow], ps[:, :ow], bp_s[:, o0:o0 + ow])
                        nc.sync.dma_start(
                            out=out[qt * 128:(qt + 1) * 128, o0:o0 + ow], in_=ot[:, :ow]
                        )

    nc.compile()
    return nc


def host_prep(q, k, v, attention_mask, Wq, Wk, Wv, Wp, bp, nq_per_core=None):
    """Pack valid keys, slice + cast full inputs into per-core input maps."""
    nq = nq_per_core or (NQ_FULL * B // N_CORES)
    bsz, nk_full = attention_mask.shape
    cores_per_b = N_CORES // bsz
    idxs = [np.flatnonzero(attention_mask[b]) for b in range(bsz)]
    nv_min = min(len(ix) for ix in idxs)
    nk = max(128, -(-max(len(ix) for ix in idxs) // 128) * 128)  # padded packed len
    nkt = nk // 128
    nbias = max(1, -(-(nk - nv_min) // 128))

    wqT = np.ascontiguousarray(Wq.T).astype(NPBF16)
    wkT = np.ascontiguousarray(Wk.T).astype(NPBF16)
    wvT = np.ascontiguousarray(Wv.T).astype(NPBF16)
    wpT = np.ascontiguousarray(Wp.T).astype(NPBF16)
    bpb = np.ascontiguousarray(bp[None, :]).astype(np.float32)

    packed = []
    for b in range(bsz):
        ix = idxs[b]
        kp = np.zeros((nk, k.shape[2]), NPBF16)
        vp = np.zeros((nk, v.shape[2]), NPBF16)
        kp[:len(ix)] = k[b][ix].astype(NPBF16)
        vp[:len(ix)] = v[b][ix].astype(NPBF16)
        mb = np.full(nk, MASK_NEG, np.float32)
        mb[:len(ix)] = 0.0
        packed.append((np.ascontiguousarray(kp.T), np.ascontiguousarray(vp.T),
                       np.ascontiguousarray(mb.reshape(nkt, 128).T)))

    in_maps = []
    for c in range(N_CORES):
        b, qi = divmod(c, cores_per_b)
        kp, vp, mb = packed[b]
        in_maps.append({
            "xq": np.ascontiguousarray(q[b, qi * nq:(qi + 1) * nq].astype(NPBF16).T),
            "xk": kp, "xv": vp,
            "wq": wqT, "wk": wkT, "wv": wvT, "wp": wpT,
            "maskb": mb, "bpb": bpb,
        })
    return in_maps, nk, nbias


_NC_CACHE = {}


def get_nc(nq, nk, nbias=2):
    key = (nq, nk, nbias)
    if key not in _NC_CACHE:
        _NC_CACHE[key] = build_nc(nq, nk, nbias)
    return _NC_CACHE[key]


def kernel(q, k, v, attention_mask, Wq, Wk, Wv, Wp, bp):
    nq = NQ_FULL * B // N_CORES
    in_maps, nk, nbias = host_prep(q, k, v, attention_mask, Wq, Wk, Wv, Wp, bp)
    nc = get_nc(nq, nk, nbias)
    res = run_bass_kernel_spmd(nc, in_maps, core_ids=list(range(N_CORES)))
    cores_per_b = N_CORES // B
    out = np.empty((B, NQ_FULL, D), np.float32)
    for c in range(N_CORES):
        b, qi = divmod(c, cores_per_b)
        out[b, qi * nq:(qi + 1) * nq] = res.results[c]["out"]
    return out
